# revision 3
# baseline (speedup 1.0000x reference)
"""Trainium2 Bass kernel v5: AB-paired chain-DAG generator MLP.

Key idea vs v4 (236 us): each PSUM bank holds ONE node for TWO chunks
("A" at partitions 0:64, "B" at 64:128), so every relu is a full
[128, 512] op using all 128 DVE/ACT lanes (v4's per-node relus used 64).
Base / chain / collect matmuls become block-diagonal single instructions
covering both chunks at once:

  per 1024 samples: 16 base [64x128] + 13 chain [128x128 block-diag]
  + 16 collect [128x32 strip-accumulate] = 45 matmuls (v4: 74) and
  16 relus + 1 biased copy (v4: 30).

PSUM: 3 slots x 2 rotating node banks + 2 rotating collect strips = 8.
Engines: relus alternate Scalar/Vector; finals on Scalar (Identity+bias).
"""

import threading

import numpy as np
import ml_dtypes

import concourse.bacc as bacc
import concourse.mybir as mybir
from concourse.bass_utils import run_bass_kernel_spmd
from concourse.tile import TileContext

N_CORES = 8
B_FULL = 131072
B_S = B_FULL // N_CORES  # 16384
CHUNK = 512
PAIR = 2 * CHUNK         # 1024 samples per pair-phase
N_PAIRS = B_S // PAIR    # 16
I_DIM = 16
I_STEPS = 16             # steps per pair-phase (one node closes per step)
NSLOT = 3
STAG = 5

F32 = mybir.dt.float32
BF16 = mybir.dt.bfloat16
BF16_NP = ml_dtypes.bfloat16


def build_nc(b_s: int = B_S, num_devices: int = N_CORES):
    n_pairs = b_s // PAIR

    nc = bacc.Bacc(
        "TRN2", target_bir_lowering=False, debug=False, num_devices=num_devices
    )

    xt_d = nc.dram_tensor("XT", [128, n_pairs * 4096], BF16, kind="ExternalInput").ap()
    px_d = nc.dram_tensor("PX", [128, 2048], BF16, kind="ExternalInput").ap()
    mc_d = nc.dram_tensor("MC", [128, 1664], BF16, kind="ExternalInput").ap()
    cl_d = nc.dram_tensor("CLW", [128, 512], BF16, kind="ExternalInput").ap()
    b2_d = nc.dram_tensor("B2", [32, 1], F32, kind="ExternalInput").ap()
    out_d = nc.dram_tensor("OUT", [16, b_s], F32, kind="ExternalOutput").ap()

    relu_f = mybir.ActivationFunctionType.Relu
    ident_f = mybir.ActivationFunctionType.Identity

    with TileContext(nc) as tc:
        with (
            tc.tile_pool(name="consts", bufs=1) as cpool,
            tc.tile_pool(name="xs", bufs=12) as xpool,
            tc.tile_pool(name="hbuf", bufs=56) as hpool,
            tc.tile_pool(name="obuf", bufs=4) as opool,
            tc.tile_pool(name="pb0", bufs=2, space="PSUM") as pbank0,
            tc.tile_pool(name="pb1", bufs=2, space="PSUM") as pbank1,
            tc.tile_pool(name="pb2", bufs=2, space="PSUM") as pbank2,
            tc.tile_pool(name="pstr", bufs=2, space="PSUM") as spool,
        ):
            bank_pools = [pbank0, pbank1, pbank2]

            px_t = cpool.tile([128, 2048], BF16)
            nc.sync.dma_start(out=px_t[:, :], in_=px_d[:, :])
            mc_t = cpool.tile([128, 1664], BF16)
            nc.sync.dma_start(out=mc_t[:, :], in_=mc_d[:, :])
            cl_t = cpool.tile([128, 512], BF16)
            nc.sync.dma_start(out=cl_t[:, :], in_=cl_d[:, :])
            b2_t = cpool.tile([32, 1], F32)
            nc.sync.dma_start(out=b2_t[:, :], in_=b2_d[:, :])

            class PairState:
                def __init__(self, g):
                    self.g = g
                    self.banks = [None] * I_DIM
                    self.h = [None] * I_DIM
                    self.strip = None

            pairs = [PairState(g) for g in range(n_pairs)]
            xtiles = {}  # (g, t) -> tile [128, 2048]

            def emit_xdma(g, t):
                x_t = xpool.tile([128, 2048], BF16, tag="x", name=f"x_{g}_{t}")
                xtiles[(g, t)] = x_t
                c0 = g * 4096 + t * 2048
                nc.sync.dma_start(out=x_t[:, :], in_=xt_d[:, c0:c0 + 2048])

            def emit_base(g, i, slot):
                st = pairs[g]
                p0 = 64 * (i % 2)
                bank = bank_pools[slot].tile(
                    [128, CHUNK], F32, tag="bank", name=f"bank_{g}_{i}"
                )
                st.banks[i] = bank
                k = i % 8
                x_t = xtiles[(g, i // 8)]
                f0 = 512 * (k // 2)
                solo = i == 0 or i > 13  # no incoming chain contribution
                nc.tensor.matmul(
                    out=bank[:, :],
                    lhsT=px_t[p0:p0 + 64, 128 * i:128 * (i + 1)],
                    rhs=x_t[p0:p0 + 64, f0:f0 + CHUNK],
                    start=True,
                    stop=solo,
                    skip_group_check=True,
                )

            def emit_relu(g, i, eng):
                st = pairs[g]
                h = hpool.tile([128, CHUNK], BF16, tag="h", name=f"h_{g}_{i}")
                st.h[i] = h
                if eng == 0:
                    nc.scalar.activation(h[:, :], st.banks[i][:, :], relu_f)
                else:
                    nc.vector.tensor_scalar_max(
                        out=h[:, :], in0=st.banks[i][:, :], scalar1=0.0
                    )
                st.banks[i] = None

            def emit_chain(g, i):
                # h_i -> preact of node i+1 (both chunks via block-diag mc)
                st = pairs[g]
                nc.tensor.matmul(
                    out=st.banks[i + 1][:, :],
                    lhsT=mc_t[:, 128 * i:128 * (i + 1)],
                    rhs=st.h[i][:, :],
                    start=False,
                    stop=True,
                    skip_group_check=True,
                )

            def make_collect_ops(g):
                st = pairs[g]
                strip = spool.tile([32, CHUNK], F32, tag="strip", name=f"strip_{g}")
                st.strip = strip

                def collect(i):
                    nc.tensor.matmul(
                        out=strip[:, :],
                        lhsT=cl_t[:, 32 * i:32 * (i + 1)],
                        rhs=st.h[i][:, :],
                        start=(i == 0),
                        stop=(i == I_DIM - 1),
                        skip_group_check=True,
                    )
                    st.h[i] = None

                ops = [lambda i=i: collect(i) for i in range(I_DIM)]

                def final():
                    o_t = opool.tile([32, CHUNK], F32, tag="o", name=f"o_{g}")
                    nc.scalar.activation(
                        o_t[:, :], strip[:, :], ident_f, bias=b2_t[:, 0:1]
                    )
                    c0 = g * PAIR
                    nc.sync.dma_start(
                        out=out_d[:, c0:c0 + CHUNK], in_=o_t[0:16, :]
                    )
                    nc.sync.dma_start(
                        out=out_d[:, c0 + CHUNK:c0 + PAIR], in_=o_t[16:32, :]
                    )
                    st.strip = None

                ops.append(final)
                return ops

            # ---------------- pipeline ----------------
            pending = []

            def slot_pos(k, T):
                q = T - STAG * k
                if q < 0:
                    return None, None
                j, i = divmod(q, I_STEPS)
                g = NSLOT * j + k
                return (g, i) if g < n_pairs else (None, None)

            # prologue: x DMAs for the first pairs in first-use order, then
            # scratch matmuls (garbage, never read) to ramp the PE clock
            for g in range(min(2, n_pairs)):
                emit_xdma(g, 0)
                emit_xdma(g, 1)
            ws0 = pbank2.tile([128, CHUNK], F32, tag="bank", name="warm0")
            ws1 = pbank2.tile([128, CHUNK], F32, tag="bank", name="warm1")
            for w in range(24):
                nc.tensor.matmul(
                    out=(ws0 if w % 2 == 0 else ws1)[:, :],
                    lhsT=px_t[(w % 2) * 64:(w % 2) * 64 + 48, 0:128],
                    rhs=px_t[(w % 2) * 64:(w % 2) * 64 + 48, 0:CHUNK],
                    start=True, stop=True, skip_group_check=True,
                )
            if n_pairs > 2:
                emit_xdma(2, 0)
                emit_xdma(2, 1)

            max_T = I_STEPS * ((n_pairs + NSLOT - 1) // NSLOT) + STAG * NSLOT + 8
            for T in range(max_T):
                for k in range(NSLOT):
                    g, i = slot_pos(k, T)
                    if g is None:
                        continue
                    # x prefetch for this slot's next pair
                    if g + NSLOT < n_pairs:
                        if i == 2:
                            emit_xdma(g + NSLOT, 0)
                        elif i == 9:
                            emit_xdma(g + NSLOT, 1)
                    # JIT bases: node i+1 before its chain lands; node 0 of
                    # this pair at phase start (first phase) or emitted at
                    # the previous pair's last step (steady state)
                    if i == 0 and g < NSLOT:
                        emit_base(g, 0, k)
                    if i < I_STEPS - 1:
                        emit_base(g, i + 1, k)
                    elif g + NSLOT < n_pairs:
                        emit_base(g + NSLOT, 0, k)
                    emit_relu(g, i, (T + k) % 2)
                    if i <= 12:
                        emit_chain(g, i)
                    npop = 2 if len(pending) > 20 else (1 if pending else 0)
                    for _ in range(npop):
                        if pending:
                            pending.pop(0)()
                    if i == I_STEPS - 1:
                        pending.extend(make_collect_ops(g))
            while pending:
                pending.pop(0)()

    nc.compile()
    return nc


def prep_weights(noise_d, mu, sigma, Wc, W1, b1, W2, b2):
    theta = mu + np.log1p(np.exp(sigma)) * noise_d  # [4, 256]
    w_p = W1[:, 48, :]  # [16, 64]
    b1e = b1.copy()
    for i in range(1, 14):
        b1e[i] = b1[i] + w_p[i] * b2[i - 1]

    px = np.zeros((128, 2048), np.float32)
    for i in range(I_DIM):
        p0 = 64 * (i % 2)
        blk = np.zeros((32, 64), np.float32)
        blk[0:10] = Wc[:, 16 * i:16 * (i + 1)] @ W1[i, 0:16, :]
        blk[10:14] = theta[:, 16 * i:16 * (i + 1)] @ W1[i, 16:32, :]
        blk[14] = b1e[i]
        blk[16:32] = W1[i, 32:48, :]
        px[p0 + 0:p0 + 32, 128 * i:128 * i + 64] = blk        # chunk A
        px[p0 + 32:p0 + 64, 128 * i + 64:128 * i + 128] = blk  # chunk B

    mc = np.zeros((128, 1664), np.float32)
    for i in range(13):
        blk = np.outer(W2[i], w_p[i + 1])
        mc[0:64, 128 * i:128 * i + 64] = blk
        mc[64:128, 128 * i + 64:128 * i + 128] = blk

    cl = np.zeros((128, 512), np.float32)
    for i in range(I_DIM):
        cl[0:64, 32 * i + i] = W2[i]
        cl[64:128, 32 * i + 16 + i] = W2[i]

    b2x = np.concatenate([b2, b2]).reshape(32, 1).astype(np.float32)

    return {
        "PX": px.astype(BF16_NP),
        "MC": mc.astype(BF16_NP),
        "CLW": cl.astype(BF16_NP),
        "B2": b2x,
    }


def prep_core_inputs(noise, input_c, input_d, c, b_s: int = B_S):
    b0, b1_ = c * b_s, (c + 1) * b_s
    s = np.zeros((16, b_s), np.float32)
    s[0:10] = input_c[b0:b1_].T
    s[10:14] = input_d[b0:b1_].T
    s[14] = 1.0
    nT = np.ascontiguousarray(noise[b0:b1_].T)
    n_pairs = b_s // PAIR
    xt = np.zeros((128, n_pairs * 4096), np.float32)
    for g in range(n_pairs):
        sA = slice(g * PAIR, g * PAIR + CHUNK)
        sB = slice(g * PAIR + CHUNK, (g + 1) * PAIR)
        for t in range(2):
            for k in range(8):
                i = 8 * t + k
                p0 = 64 * (k % 2)
                c0 = g * 4096 + t * 2048 + (k // 2) * 512
                xt[p0 + 0:p0 + 16, c0:c0 + 512] = s[:, sA]
                xt[p0 + 16:p0 + 32, c0:c0 + 512] = nT[16 * i:16 * (i + 1), sA]
                xt[p0 + 32:p0 + 48, c0:c0 + 512] = s[:, sB]
                xt[p0 + 48:p0 + 64, c0:c0 + 512] = nT[16 * i:16 * (i + 1), sB]
    return {"XT": xt.astype(BF16_NP)}


_NC_LOCK = threading.Lock()
_NC_CACHE = {}


def _get_nc():
    with _NC_LOCK:
        if "nc" not in _NC_CACHE:
            _NC_CACHE["nc"] = build_nc()
        return _NC_CACHE["nc"]


def kernel(noise, input_c, input_d, noise_d, mu, sigma, Wc, W1, b1, W2, b2):
    noise = np.asarray(noise, np.float32)
    input_c = np.asarray(input_c, np.float32)
    input_d = np.asarray(input_d, np.float32)
    w = prep_weights(
        np.asarray(noise_d, np.float32),
        np.asarray(mu, np.float32),
        np.asarray(sigma, np.float32),
        np.asarray(Wc, np.float32),
        np.asarray(W1, np.float32),
        np.asarray(b1, np.float32),
        np.asarray(W2, np.float32),
        np.asarray(b2, np.float32),
    )
    in_maps = []
    for c in range(N_CORES):
        m = prep_core_inputs(noise, input_c, input_d, c)
        m.update(w)
        in_maps.append(m)

    nc = _get_nc()
    res = run_bass_kernel_spmd(nc, in_maps, list(range(N_CORES)))
    out = np.concatenate(
        [res.results[c]["OUT"].T for c in range(N_CORES)], axis=0
    )
    return np.ascontiguousarray(out, np.float32)


# revision 13
# speedup vs baseline: 1.3427x; 1.3427x over previous
"""Trainium2 Bass kernel v5: AB-paired chain-DAG generator MLP.

Key idea vs v4 (236 us): each PSUM bank holds ONE node for TWO chunks
("A" at partitions 0:64, "B" at 64:128), so every relu is a full
[128, 512] op using all 128 DVE/ACT lanes (v4's per-node relus used 64).
Base / chain / collect matmuls become block-diagonal single instructions
covering both chunks at once:

  per 1024 samples: 16 base [64x128] + 13 chain [128x128 block-diag]
  + 16 collect [128x32 strip-accumulate] = 45 matmuls (v4: 74) and
  16 relus + 1 biased copy (v4: 30).

PSUM: 3 slots x 2 rotating node banks + 2 rotating collect strips = 8.
Engines: relus alternate Scalar/Vector; finals on Scalar (Identity+bias).
"""

import threading

import numpy as np
import ml_dtypes

import concourse.bacc as bacc
import concourse.mybir as mybir
from concourse.bass_utils import run_bass_kernel_spmd
from concourse.tile import TileContext

N_CORES = 8
B_FULL = 131072
B_S = B_FULL // N_CORES  # 16384
CHUNK = 512
PAIR = 2 * CHUNK         # 1024 samples per pair-phase
N_PAIRS = B_S // PAIR    # 16
I_DIM = 16
I_STEPS = 16             # steps per pair-phase (one node closes per step)
NSLOT = 3
STAG = 5

F32 = mybir.dt.float32
BF16 = mybir.dt.bfloat16
BF16_NP = ml_dtypes.bfloat16


def build_nc(b_s: int = B_S, num_devices: int = N_CORES):
    n_pairs = b_s // PAIR

    nc = bacc.Bacc(
        "TRN2", target_bir_lowering=False, debug=False, num_devices=num_devices
    )

    xt_d = nc.dram_tensor("XT", [128, n_pairs * 4096], BF16, kind="ExternalInput").ap()
    px_d = nc.dram_tensor("PX", [128, 2048], BF16, kind="ExternalInput").ap()
    mc_d = nc.dram_tensor("MC", [128, 1664], BF16, kind="ExternalInput").ap()
    cl_d = nc.dram_tensor("CLW", [128, 2048], BF16, kind="ExternalInput").ap()
    b2_d = nc.dram_tensor("B2", [48, 1], F32, kind="ExternalInput").ap()
    out_d = nc.dram_tensor("OUT", [16, b_s], F32, kind="ExternalOutput").ap()

    relu_f = mybir.ActivationFunctionType.Relu
    ident_f = mybir.ActivationFunctionType.Identity

    with TileContext(nc) as tc:
        with (
            tc.tile_pool(name="consts", bufs=1) as cpool,
            tc.tile_pool(name="xs", bufs=12) as xpool,
            tc.tile_pool(name="hbuf", bufs=56) as hpool,
            tc.tile_pool(name="obuf", bufs=4) as opool,
            tc.tile_pool(name="pb0", bufs=2, space="PSUM") as pbank0,
            tc.tile_pool(name="pb1", bufs=2, space="PSUM") as pbank1,
            tc.tile_pool(name="pb2", bufs=2, space="PSUM") as pbank2,
            tc.tile_pool(name="pstr", bufs=2, space="PSUM") as spool,
        ):
            bank_pools = [pbank0, pbank1, pbank2]

            px_t = cpool.tile([128, 2048], BF16)
            nc.sync.dma_start(out=px_t[:, :], in_=px_d[:, :])
            mc_t = cpool.tile([128, 1664], BF16)
            nc.sync.dma_start(out=mc_t[:, :], in_=mc_d[:, :])
            cl_t = cpool.tile([128, 2048], BF16)
            nc.sync.dma_start(out=cl_t[:, :], in_=cl_d[:, :])
            b2_t = cpool.tile([48, 1], F32)
            nc.sync.dma_start(out=b2_t[:, :], in_=b2_d[:, :])

            class PairState:
                def __init__(self, g):
                    self.g = g
                    self.banks = [None] * I_DIM
                    self.h = [None] * I_DIM
                    self.strip = None

            pairs = [PairState(g) for g in range(n_pairs)]
            xtiles = {}  # (g, t) -> tile [128, 2048]

            def emit_xdma(g, t):
                x_t = xpool.tile([128, 2048], BF16, tag="x", name=f"x_{g}_{t}")
                xtiles[(g, t)] = x_t
                c0 = g * 4096 + t * 2048
                nc.sync.dma_start(out=x_t[:, :], in_=xt_d[:, c0:c0 + 2048])

            def emit_base(g, i, slot):
                st = pairs[g]
                p0 = 64 * (i % 2)
                bank = bank_pools[slot].tile(
                    [128, CHUNK], F32, tag="bank", name=f"bank_{g}_{i}"
                )
                st.banks[i] = bank
                k = i % 8
                x_t = xtiles[(g, i // 8)]
                f0 = 512 * (k // 2)
                solo = i == 0 or i > 13  # no incoming chain contribution
                # (128,128) tile: rows outside this node's x block hit zero
                # weight rows in px, so the full-partition contraction is safe
                nc.tensor.matmul(
                    out=bank[:, :],
                    lhsT=px_t[:, 128 * i:128 * (i + 1)],
                    rhs=x_t[:, f0:f0 + CHUNK],
                    start=True,
                    stop=solo,
                    skip_group_check=True,
                )

            def emit_relu(g, i, eng):
                st = pairs[g]
                h = hpool.tile([128, CHUNK], BF16, tag="h", name=f"h_{g}_{i}")
                st.h[i] = h
                if eng == 0:
                    nc.scalar.activation(h[:, :], st.banks[i][:, :], relu_f)
                else:
                    nc.vector.tensor_scalar_max(
                        out=h[:, :], in0=st.banks[i][:, :], scalar1=0.0
                    )
                st.banks[i] = None

            def emit_chain(g, i):
                # h_i -> preact of node i+1, both chunks in one (128,128) mm
                # via the block-diagonal mc weights
                st = pairs[g]
                nc.tensor.matmul(
                    out=st.banks[i + 1][:, :],
                    lhsT=mc_t[:, 128 * i:128 * (i + 1)],
                    rhs=st.h[i][:, :],
                    start=False,
                    stop=True,
                    skip_group_check=True,
                )

            def make_collect_ops(g):
                st = pairs[g]
                strip = spool.tile([128, CHUNK], F32, tag="strip", name=f"strip_{g}")
                st.strip = strip

                def collect(i):
                    # (128,128) mm: both chunk halves of h_i -> o rows i / 32+i
                    nc.tensor.matmul(
                        out=strip[:, :],
                        lhsT=cl_t[:, 128 * i:128 * (i + 1)],
                        rhs=st.h[i][:, :],
                        start=(i == 0),
                        stop=(i == I_DIM - 1),
                        skip_group_check=True,
                    )
                    st.h[i] = None

                ops = [lambda i=i: collect(i) for i in range(I_DIM)]

                def final():
                    o_t = opool.tile([48, CHUNK], F32, tag="o", name=f"o_{g}")
                    nc.scalar.activation(
                        o_t[:, :], strip[0:48, :], ident_f, bias=b2_t[:, 0:1]
                    )
                    c0 = g * PAIR
                    nc.sync.dma_start(
                        out=out_d[:, c0:c0 + CHUNK], in_=o_t[0:16, :]
                    )
                    nc.sync.dma_start(
                        out=out_d[:, c0 + CHUNK:c0 + PAIR], in_=o_t[32:48, :]
                    )
                    st.strip = None

                ops.append(final)
                return ops

            # ---------------- pipeline ----------------
            pending = []

            def slot_pos(k, T):
                q = T - STAG * k
                if q < 0:
                    return None, None
                j, i = divmod(q, I_STEPS)
                g = NSLOT * j + k
                return (g, i) if g < n_pairs else (None, None)

            # prologue: x DMAs for the first pairs in first-use order, then
            # scratch matmuls (garbage, never read) to ramp the PE clock
            for g in range(min(2, n_pairs)):
                emit_xdma(g, 0)
                emit_xdma(g, 1)
            ws0 = pbank2.tile([128, CHUNK], F32, tag="bank", name="warm0")
            ws1 = pbank2.tile([128, CHUNK], F32, tag="bank", name="warm1")
            for w in range(24):
                nc.tensor.matmul(
                    out=(ws0 if w % 2 == 0 else ws1)[:, :],
                    lhsT=px_t[:, 0:128],
                    rhs=px_t[:, 0:CHUNK],
                    start=True, stop=True, skip_group_check=True,
                )
            if n_pairs > 2:
                emit_xdma(2, 0)
                emit_xdma(2, 1)

            max_T = I_STEPS * ((n_pairs + NSLOT - 1) // NSLOT) + STAG * NSLOT + 8
            for T in range(max_T):
                for k in range(NSLOT):
                    g, i = slot_pos(k, T)
                    if g is None:
                        continue
                    # x prefetch for this slot's next pair
                    if g + NSLOT < n_pairs:
                        if i == 2:
                            emit_xdma(g + NSLOT, 0)
                        elif i == 9:
                            emit_xdma(g + NSLOT, 1)
                    # JIT bases: node i+1 before its chain lands; node 0 of
                    # this pair at phase start (first phase) or emitted at
                    # the previous pair's last step (steady state)
                    if i == 0 and g < NSLOT:
                        emit_base(g, 0, k)
                    if i < I_STEPS - 1:
                        emit_base(g, i + 1, k)
                    elif g + NSLOT < n_pairs:
                        emit_base(g + NSLOT, 0, k)
                    # ready collect mms go on the PE queue BEFORE the chain
                    # (which stalls on the fresh relu) to avoid head-of-line
                    npop = 3 if len(pending) > 30 else (2 if pending else 0)
                    for _ in range(npop):
                        if pending:
                            pending.pop(0)()
                    emit_relu(g, i, (T + k) % 2)
                    if i <= 12:
                        emit_chain(g, i)
                    if i == I_STEPS - 1:
                        pending.extend(make_collect_ops(g))
            while pending:
                pending.pop(0)()

    nc.compile()
    return nc


def prep_weights(noise_d, mu, sigma, Wc, W1, b1, W2, b2):
    theta = mu + np.log1p(np.exp(sigma)) * noise_d  # [4, 256]
    w_p = W1[:, 48, :]  # [16, 64]
    b1e = b1.copy()
    for i in range(1, 14):
        b1e[i] = b1[i] + w_p[i] * b2[i - 1]

    px = np.zeros((128, 2048), np.float32)
    for i in range(I_DIM):
        p0 = 64 * (i % 2)
        blk = np.zeros((32, 64), np.float32)
        blk[0:10] = Wc[:, 16 * i:16 * (i + 1)] @ W1[i, 0:16, :]
        blk[10:14] = theta[:, 16 * i:16 * (i + 1)] @ W1[i, 16:32, :]
        blk[14] = b1e[i]
        blk[16:32] = W1[i, 32:48, :]
        px[p0 + 0:p0 + 32, 128 * i:128 * i + 64] = blk        # chunk A
        px[p0 + 32:p0 + 64, 128 * i + 64:128 * i + 128] = blk  # chunk B

    mc = np.zeros((128, 1664), np.float32)
    for i in range(13):
        blk = np.outer(W2[i], w_p[i + 1])
        mc[0:64, 128 * i:128 * i + 64] = blk
        mc[64:128, 128 * i + 64:128 * i + 128] = blk

    cl = np.zeros((128, 2048), np.float32)
    for i in range(I_DIM):
        cl[0:64, 128 * i + i] = W2[i]         # chunk A -> strip row i
        cl[64:128, 128 * i + 32 + i] = W2[i]  # chunk B -> strip row 32+i

    b2x = np.zeros((48, 1), np.float32)
    b2x[0:16, 0] = b2
    b2x[32:48, 0] = b2

    return {
        "PX": px.astype(BF16_NP),
        "MC": mc.astype(BF16_NP),
        "CLW": cl.astype(BF16_NP),
        "B2": b2x,
    }


def prep_core_inputs(noise, input_c, input_d, c, b_s: int = B_S):
    b0, b1_ = c * b_s, (c + 1) * b_s
    s = np.zeros((16, b_s), np.float32)
    s[0:10] = input_c[b0:b1_].T
    s[10:14] = input_d[b0:b1_].T
    s[14] = 1.0
    nT = np.ascontiguousarray(noise[b0:b1_].T)
    n_pairs = b_s // PAIR
    xt = np.zeros((128, n_pairs * 4096), np.float32)
    for g in range(n_pairs):
        sA = slice(g * PAIR, g * PAIR + CHUNK)
        sB = slice(g * PAIR + CHUNK, (g + 1) * PAIR)
        for t in range(2):
            for k in range(8):
                i = 8 * t + k
                p0 = 64 * (k % 2)
                c0 = g * 4096 + t * 2048 + (k // 2) * 512
                xt[p0 + 0:p0 + 16, c0:c0 + 512] = s[:, sA]
                xt[p0 + 16:p0 + 32, c0:c0 + 512] = nT[16 * i:16 * (i + 1), sA]
                xt[p0 + 32:p0 + 48, c0:c0 + 512] = s[:, sB]
                xt[p0 + 48:p0 + 64, c0:c0 + 512] = nT[16 * i:16 * (i + 1), sB]
    return {"XT": xt.astype(BF16_NP)}


_NC_LOCK = threading.Lock()
_NC_CACHE = {}


def _get_nc():
    with _NC_LOCK:
        if "nc" not in _NC_CACHE:
            _NC_CACHE["nc"] = build_nc()
        return _NC_CACHE["nc"]


def kernel(noise, input_c, input_d, noise_d, mu, sigma, Wc, W1, b1, W2, b2):
    noise = np.asarray(noise, np.float32)
    input_c = np.asarray(input_c, np.float32)
    input_d = np.asarray(input_d, np.float32)
    w = prep_weights(
        np.asarray(noise_d, np.float32),
        np.asarray(mu, np.float32),
        np.asarray(sigma, np.float32),
        np.asarray(Wc, np.float32),
        np.asarray(W1, np.float32),
        np.asarray(b1, np.float32),
        np.asarray(W2, np.float32),
        np.asarray(b2, np.float32),
    )
    in_maps = []
    for c in range(N_CORES):
        m = prep_core_inputs(noise, input_c, input_d, c)
        m.update(w)
        in_maps.append(m)

    nc = _get_nc()
    res = run_bass_kernel_spmd(nc, in_maps, list(range(N_CORES)))
    out = np.concatenate(
        [res.results[c]["OUT"].T for c in range(N_CORES)], axis=0
    )
    return np.ascontiguousarray(out, np.float32)


# revision 15
# speedup vs baseline: 1.3550x; 1.0092x over previous
"""Trainium2 Bass kernel v5: AB-paired chain-DAG generator MLP.

Key idea vs v4 (236 us): each PSUM bank holds ONE node for TWO chunks
("A" at partitions 0:64, "B" at 64:128), so every relu is a full
[128, 512] op using all 128 DVE/ACT lanes (v4's per-node relus used 64).
Base / chain / collect matmuls become block-diagonal single instructions
covering both chunks at once:

  per 1024 samples: 16 base [64x128] + 13 chain [128x128 block-diag]
  + 16 collect [128x32 strip-accumulate] = 45 matmuls (v4: 74) and
  16 relus + 1 biased copy (v4: 30).

PSUM: 3 slots x 2 rotating node banks + 2 rotating collect strips = 8.
Engines: relus alternate Scalar/Vector; finals on Scalar (Identity+bias).
"""

import threading

import numpy as np
import ml_dtypes

import concourse.bacc as bacc
import concourse.mybir as mybir
from concourse.bass_utils import run_bass_kernel_spmd
from concourse.tile import TileContext

N_CORES = 8
B_FULL = 131072
B_S = B_FULL // N_CORES  # 16384
CHUNK = 512
PAIR = 2 * CHUNK         # 1024 samples per pair-phase
N_PAIRS = B_S // PAIR    # 16
I_DIM = 16
I_STEPS = 16             # steps per pair-phase (one node closes per step)
NSLOT = 3
STAG = 5

F32 = mybir.dt.float32
BF16 = mybir.dt.bfloat16
BF16_NP = ml_dtypes.bfloat16


def build_nc(b_s: int = B_S, num_devices: int = N_CORES):
    n_pairs = b_s // PAIR

    nc = bacc.Bacc(
        "TRN2", target_bir_lowering=False, debug=False, num_devices=num_devices
    )

    xt_d = nc.dram_tensor("XT", [128, n_pairs * 4096], BF16, kind="ExternalInput").ap()
    px_d = nc.dram_tensor("PX", [128, 2048], BF16, kind="ExternalInput").ap()
    mc_d = nc.dram_tensor("MC", [128, 1664], BF16, kind="ExternalInput").ap()
    cl_d = nc.dram_tensor("CLW", [128, 2048], BF16, kind="ExternalInput").ap()
    b2_d = nc.dram_tensor("B2", [48, 1], F32, kind="ExternalInput").ap()
    out_d = nc.dram_tensor("OUT", [16, b_s], F32, kind="ExternalOutput").ap()

    relu_f = mybir.ActivationFunctionType.Relu
    ident_f = mybir.ActivationFunctionType.Identity

    with TileContext(nc) as tc:
        with (
            tc.tile_pool(name="consts", bufs=1) as cpool,
            tc.tile_pool(name="xs", bufs=12) as xpool,
            tc.tile_pool(name="hbuf", bufs=56) as hpool,
            tc.tile_pool(name="obuf", bufs=4) as opool,
            tc.tile_pool(name="pb0", bufs=2, space="PSUM") as pbank0,
            tc.tile_pool(name="pb1", bufs=2, space="PSUM") as pbank1,
            tc.tile_pool(name="pb2", bufs=2, space="PSUM") as pbank2,
            tc.tile_pool(name="pstr", bufs=2, space="PSUM") as spool,
        ):
            bank_pools = [pbank0, pbank1, pbank2]

            px_t = cpool.tile([128, 2048], BF16)
            nc.sync.dma_start(out=px_t[:, :], in_=px_d[:, :])
            mc_t = cpool.tile([128, 1664], BF16)
            nc.sync.dma_start(out=mc_t[:, :], in_=mc_d[:, :])
            cl_t = cpool.tile([128, 2048], BF16)
            nc.sync.dma_start(out=cl_t[:, :], in_=cl_d[:, :])
            b2_t = cpool.tile([48, 1], F32)
            nc.sync.dma_start(out=b2_t[:, :], in_=b2_d[:, :])

            class PairState:
                def __init__(self, g):
                    self.g = g
                    self.banks = [None] * I_DIM
                    self.h = [None] * I_DIM
                    self.strip = None

            pairs = [PairState(g) for g in range(n_pairs)]
            xtiles = {}  # (g, t) -> tile [128, 2048]

            def emit_xdma(g, t):
                x_t = xpool.tile([128, 2048], BF16, tag="x", name=f"x_{g}_{t}")
                xtiles[(g, t)] = x_t
                c0 = g * 4096 + t * 2048
                nc.sync.dma_start(out=x_t[:, :], in_=xt_d[:, c0:c0 + 2048])

            def emit_base(g, i, slot):
                st = pairs[g]
                p0 = 64 * (i % 2)
                bank = bank_pools[slot].tile(
                    [128, CHUNK], F32, tag="bank", name=f"bank_{g}_{i}"
                )
                st.banks[i] = bank
                k = i % 8
                x_t = xtiles[(g, i // 8)]
                f0 = 512 * (k // 2)
                solo = i == 0 or i > 13  # no incoming chain contribution
                # (128,128) tile: rows outside this node's x block hit zero
                # weight rows in px, so the full-partition contraction is safe
                nc.tensor.matmul(
                    out=bank[:, :],
                    lhsT=px_t[:, 128 * i:128 * (i + 1)],
                    rhs=x_t[:, f0:f0 + CHUNK],
                    start=True,
                    stop=solo,
                    skip_group_check=True,
                )

            def emit_relu(g, i, eng):
                st = pairs[g]
                h = hpool.tile([128, CHUNK], BF16, tag="h", name=f"h_{g}_{i}")
                st.h[i] = h
                if eng == 0:
                    nc.scalar.activation(h[:, :], st.banks[i][:, :], relu_f)
                else:
                    nc.vector.tensor_scalar_max(
                        out=h[:, :], in0=st.banks[i][:, :], scalar1=0.0
                    )
                st.banks[i] = None

            def emit_chain(g, i):
                # h_i -> preact of node i+1, both chunks in one (128,128) mm
                # via the block-diagonal mc weights
                st = pairs[g]
                nc.tensor.matmul(
                    out=st.banks[i + 1][:, :],
                    lhsT=mc_t[:, 128 * i:128 * (i + 1)],
                    rhs=st.h[i][:, :],
                    start=False,
                    stop=True,
                    skip_group_check=True,
                )

            def make_collect_ops(g):
                st = pairs[g]
                strip = spool.tile([128, CHUNK], F32, tag="strip", name=f"strip_{g}")
                st.strip = strip

                def collect(i):
                    # (128,128) mm: both chunk halves of h_i -> o rows i / 32+i
                    nc.tensor.matmul(
                        out=strip[:, :],
                        lhsT=cl_t[:, 128 * i:128 * (i + 1)],
                        rhs=st.h[i][:, :],
                        start=(i == 0),
                        stop=(i == I_DIM - 1),
                        skip_group_check=True,
                    )
                    st.h[i] = None

                ops = [lambda i=i: collect(i) for i in range(I_DIM)]

                def final():
                    o_t = opool.tile([48, CHUNK], F32, tag="o", name=f"o_{g}")
                    nc.scalar.activation(
                        o_t[:, :], strip[0:48, :], ident_f, bias=b2_t[:, 0:1]
                    )
                    c0 = g * PAIR
                    nc.sync.dma_start(
                        out=out_d[:, c0:c0 + CHUNK], in_=o_t[0:16, :]
                    )
                    nc.sync.dma_start(
                        out=out_d[:, c0 + CHUNK:c0 + PAIR], in_=o_t[32:48, :]
                    )
                    st.strip = None

                ops.append(final)
                return ops

            # ---------------- pipeline ----------------
            pending = []

            def slot_pos(k, T):
                q = T - STAG * k
                if q < 0:
                    return None, None
                j, i = divmod(q, I_STEPS)
                g = NSLOT * j + k
                return (g, i) if g < n_pairs else (None, None)

            # prologue: x DMAs for the first pairs in first-use order, then
            # scratch matmuls (garbage, never read) to ramp the PE clock
            for g in range(min(2, n_pairs)):
                emit_xdma(g, 0)
                emit_xdma(g, 1)
            # clock-ramp warmup on an uninitialized scratch tile: no DMA
            # dependency, so the PE spins up while the const/x DMAs land
            wsrc = cpool.tile([128, CHUNK], BF16)
            nc.gpsimd.memset(wsrc[:, :], 0.0)
            ws0 = pbank2.tile([128, CHUNK], F32, tag="bank", name="warm0")
            ws1 = pbank2.tile([128, CHUNK], F32, tag="bank", name="warm1")
            for w in range(14):
                nc.tensor.matmul(
                    out=(ws0 if w % 2 == 0 else ws1)[:, :],
                    lhsT=wsrc[:, 0:128],
                    rhs=wsrc[:, 0:CHUNK],
                    start=True, stop=True, skip_group_check=True,
                )
            if n_pairs > 2:
                emit_xdma(2, 0)
                emit_xdma(2, 1)

            max_T = I_STEPS * ((n_pairs + NSLOT - 1) // NSLOT) + STAG * NSLOT + 8
            for T in range(max_T):
                for k in range(NSLOT):
                    g, i = slot_pos(k, T)
                    if g is None:
                        continue
                    # x prefetch for this slot's next pair
                    if g + NSLOT < n_pairs:
                        if i == 2:
                            emit_xdma(g + NSLOT, 0)
                        elif i == 9:
                            emit_xdma(g + NSLOT, 1)
                    # JIT bases: node i+1 before its chain lands; node 0 of
                    # this pair at phase start (first phase) or emitted at
                    # the previous pair's last step (steady state)
                    if i == 0 and g < NSLOT:
                        emit_base(g, 0, k)
                    if i < I_STEPS - 1:
                        emit_base(g, i + 1, k)
                    elif g + NSLOT < n_pairs:
                        emit_base(g + NSLOT, 0, k)
                    # ready collect mms go on the PE queue BEFORE the chain
                    # (which stalls on the fresh relu) to avoid head-of-line
                    npop = 3 if len(pending) > 30 else (2 if pending else 0)
                    for _ in range(npop):
                        if pending:
                            pending.pop(0)()
                    emit_relu(g, i, (T + k) % 2)
                    if i <= 12:
                        emit_chain(g, i)
                    if i == I_STEPS - 1:
                        pending.extend(make_collect_ops(g))
            while pending:
                pending.pop(0)()

    nc.compile()
    return nc


def prep_weights(noise_d, mu, sigma, Wc, W1, b1, W2, b2):
    theta = mu + np.log1p(np.exp(sigma)) * noise_d  # [4, 256]
    w_p = W1[:, 48, :]  # [16, 64]
    b1e = b1.copy()
    for i in range(1, 14):
        b1e[i] = b1[i] + w_p[i] * b2[i - 1]

    px = np.zeros((128, 2048), np.float32)
    for i in range(I_DIM):
        p0 = 64 * (i % 2)
        blk = np.zeros((32, 64), np.float32)
        blk[0:10] = Wc[:, 16 * i:16 * (i + 1)] @ W1[i, 0:16, :]
        blk[10:14] = theta[:, 16 * i:16 * (i + 1)] @ W1[i, 16:32, :]
        blk[14] = b1e[i]
        blk[16:32] = W1[i, 32:48, :]
        px[p0 + 0:p0 + 32, 128 * i:128 * i + 64] = blk        # chunk A
        px[p0 + 32:p0 + 64, 128 * i + 64:128 * i + 128] = blk  # chunk B

    mc = np.zeros((128, 1664), np.float32)
    for i in range(13):
        blk = np.outer(W2[i], w_p[i + 1])
        mc[0:64, 128 * i:128 * i + 64] = blk
        mc[64:128, 128 * i + 64:128 * i + 128] = blk

    cl = np.zeros((128, 2048), np.float32)
    for i in range(I_DIM):
        cl[0:64, 128 * i + i] = W2[i]         # chunk A -> strip row i
        cl[64:128, 128 * i + 32 + i] = W2[i]  # chunk B -> strip row 32+i

    b2x = np.zeros((48, 1), np.float32)
    b2x[0:16, 0] = b2
    b2x[32:48, 0] = b2

    return {
        "PX": px.astype(BF16_NP),
        "MC": mc.astype(BF16_NP),
        "CLW": cl.astype(BF16_NP),
        "B2": b2x,
    }


def prep_core_inputs(noise, input_c, input_d, c, b_s: int = B_S):
    b0, b1_ = c * b_s, (c + 1) * b_s
    s = np.zeros((16, b_s), np.float32)
    s[0:10] = input_c[b0:b1_].T
    s[10:14] = input_d[b0:b1_].T
    s[14] = 1.0
    nT = np.ascontiguousarray(noise[b0:b1_].T)
    n_pairs = b_s // PAIR
    xt = np.zeros((128, n_pairs * 4096), np.float32)
    for g in range(n_pairs):
        sA = slice(g * PAIR, g * PAIR + CHUNK)
        sB = slice(g * PAIR + CHUNK, (g + 1) * PAIR)
        for t in range(2):
            for k in range(8):
                i = 8 * t + k
                p0 = 64 * (k % 2)
                c0 = g * 4096 + t * 2048 + (k // 2) * 512
                xt[p0 + 0:p0 + 16, c0:c0 + 512] = s[:, sA]
                xt[p0 + 16:p0 + 32, c0:c0 + 512] = nT[16 * i:16 * (i + 1), sA]
                xt[p0 + 32:p0 + 48, c0:c0 + 512] = s[:, sB]
                xt[p0 + 48:p0 + 64, c0:c0 + 512] = nT[16 * i:16 * (i + 1), sB]
    return {"XT": xt.astype(BF16_NP)}


_NC_LOCK = threading.Lock()
_NC_CACHE = {}


def _get_nc():
    with _NC_LOCK:
        if "nc" not in _NC_CACHE:
            _NC_CACHE["nc"] = build_nc()
        return _NC_CACHE["nc"]


def kernel(noise, input_c, input_d, noise_d, mu, sigma, Wc, W1, b1, W2, b2):
    noise = np.asarray(noise, np.float32)
    input_c = np.asarray(input_c, np.float32)
    input_d = np.asarray(input_d, np.float32)
    w = prep_weights(
        np.asarray(noise_d, np.float32),
        np.asarray(mu, np.float32),
        np.asarray(sigma, np.float32),
        np.asarray(Wc, np.float32),
        np.asarray(W1, np.float32),
        np.asarray(b1, np.float32),
        np.asarray(W2, np.float32),
        np.asarray(b2, np.float32),
    )
    in_maps = []
    for c in range(N_CORES):
        m = prep_core_inputs(noise, input_c, input_d, c)
        m.update(w)
        in_maps.append(m)

    nc = _get_nc()
    res = run_bass_kernel_spmd(nc, in_maps, list(range(N_CORES)))
    out = np.concatenate(
        [res.results[c]["OUT"].T for c in range(N_CORES)], axis=0
    )
    return np.ascontiguousarray(out, np.float32)
